# revision 19
# baseline (speedup 1.0000x reference)
"""Trainium2 Bass kernel for a type-dispatched GNN message-passing layer.

Strategy (8 NeuronCores, SPMD, no collectives):
  - Nodes are relabeled host-side and assigned to cores/windows so that every
    256-node window is node-type-pure and every (window, col-type) cell has a
    balanced edge count (greedy 2D packing).  Each core owns 2560 local node
    slots (2500 real nodes + holes) and all edges whose source (row) node it
    owns, so the segment-sum is core-local: no cross-core reduction at all.
  - Per core: 10 windows x 2 cells x Q tiles of 128 edge slots.  The pair
    identity of each cell is resolved host-side into a per-cell weight block
    DMA'd from DRAM, so the instruction stream is identical on all cores.
  - x is gathered per edge with dma_gather(transpose=True) in bf16, landing
    feature-on-partition, ready to be the moving matmul operand.
  - Edge MLP: L1 weights-stationary -> h1T [h,e]; L2 h1-stationary -> ef [e,k];
    scatter via a per-tile one-hot S [e,256] built on DVE (iota == row-offset),
    accumulated in PSUM per window -> aggT [h,n].
  - Node MLP computes both types and blends with a predicate mask; the f32
    residual x0 is added exactly.
"""

import numpy as np
import ml_dtypes

N, F, E, FE, H, OUT = 20000, 128, 320000, 16, 128, 128
NTYPES, NPAIRS = 2, 4
C = 8                  # cores
NLOC = N // C          # real nodes per core
WIN = 256              # node window (scatter S width)
NWIN = 10              # windows per core (5 per node type)
NWT = NWIN // 2        # windows per type
LCAP = NWIN * WIN      # local node slots per core (2560)
NCELL = NWIN * 2       # cells per core
Q = 16                 # tiles of 128 edge slots per cell
CELL = Q * 128         # 2048 edge slots per cell
ECAP = NCELL * CELL    # 40960 edge slots per core
T = ECAP // 128        # 320 tiles per core
SIDX = ECAP // 16      # idx columns (2560)
KEA = FE + 1           # ea chunk rows incl. bias row
WCOLS = 4 * H          # weight block free dim: W1a|W1b|W2|W1c side by side

BF16 = ml_dtypes.bfloat16

_STATE: dict = {}


def _pack_nodes(row, col, nt):
    """Assign nodes to (core, window, slot); windows are type-pure and
    edge-load balanced.  Returns core_of[n], loc_of[n] (0..LCAP-1), and the
    per-core local->original node map (-1 for holes)."""
    deg = np.zeros((N, 2), np.int64)
    np.add.at(deg, (row, nt[col]), 1)
    degt = deg.sum(1)

    core_of = np.full(N, -1, np.int64)
    loc_of = np.full(N, -1, np.int64)
    node_at = np.full((C, LCAP), -1, np.int64)

    for t in (0, 1):
        ids = np.where(nt == t)[0]
        ids = ids[np.argsort(-degt[ids], kind="stable")]
        # snake-deal across cores to balance total degree
        percore = [[] for _ in range(C)]
        for i, n in enumerate(ids):
            c = i % (2 * C)
            percore[c if c < C else 2 * C - 1 - c].append(n)
        for c in range(C):
            tn = percore[c]
            assert len(tn) <= NWT * WIN, (t, c, len(tn))
            # greedy: place node in the type-t window minimizing the max
            # resulting cell load (2D vector packing)
            loads = np.zeros((NWT, 2))
            counts = np.zeros(NWT, np.int64)
            wbase = 0 if t == 0 else NWT
            for n in tn:  # already degree-descending
                best, bw = None, None
                for w in range(NWT):
                    if counts[w] >= WIN:
                        continue
                    m = max(loads[w][0] + deg[n][0], loads[w][1] + deg[n][1])
                    if best is None or m < best:
                        best, bw = m, w
                w = wbase + bw
                loc = w * WIN + counts[bw]
                core_of[n] = c
                loc_of[n] = loc
                node_at[c, loc] = n
                loads[bw] += deg[n]
                counts[bw] += 1
    return core_of, loc_of, node_at


def _wrap_idx(a):
    """[ECAP] int -> dma_gather idx layout [128, SIDX] int16."""
    a16 = a.reshape(SIDX, 16).T.astype(np.int16)  # element i at [i%16, i//16]
    return np.tile(a16, (8, 1))


def _prep(inputs):
    x = np.asarray(inputs["x"], np.float32)
    ei = np.asarray(inputs["edge_index"], np.int64)
    ea = np.asarray(inputs["edge_attr"], np.float32)
    nt = np.asarray(inputs["node_type"], np.int64)
    ew1 = np.asarray(inputs["ew1"], np.float32)
    eb1 = np.asarray(inputs["eb1"], np.float32)
    ew2 = np.asarray(inputs["ew2"], np.float32)
    eb2 = np.asarray(inputs["eb2"], np.float32)
    nw1 = np.asarray(inputs["nw1"], np.float32)
    nb1 = np.asarray(inputs["nb1"], np.float32)
    nw2 = np.asarray(inputs["nw2"], np.float32)
    nb2 = np.asarray(inputs["nb2"], np.float32)

    assert not eb2.any(), "nonzero eb2 not supported by this kernel layout"

    x0 = x[:, 0]                          # [N, F]
    row, col = ei[0], ei[1]
    core_of, loc_of, node_at = _pack_nodes(row, col, nt)

    # ---- edge slot layout ----
    ecore = core_of[row]
    lrow = loc_of[row]
    w_of_e = lrow // WIN
    cell = w_of_e * 2 + nt[col]           # 0..19
    key = ecore * NCELL + cell
    order = np.argsort(key, kind="stable")
    counts = np.bincount(key, minlength=C * NCELL)
    assert counts.max() <= CELL, f"cell overflow: {counts.max()} > {CELL}"

    slot_of = np.empty(E, np.int64)
    starts = np.arange(C * NCELL) * CELL
    ends = starts + counts
    pos = np.concatenate([np.arange(s, s + c) for s, c in zip(starts, counts)])
    slot_of[order] = pos                  # global slot in [0, C*ECAP)

    row_idx = np.zeros((C, ECAP), np.int64)
    col_idx = np.zeros((C, ECAP), np.int64)
    ea_pad = np.zeros((C, ECAP, KEA), np.float32)
    rmb = np.full((C, ECAP), -100.0, np.float32)
    sc, ss = slot_of // ECAP, slot_of % ECAP
    row_idx[sc, ss] = row
    col_idx[sc, ss] = col
    ea_pad[sc, ss, :FE] = ea
    ea_pad[sc, ss, FE] = 1.0
    rmb[sc, ss] = lrow % WIN

    # ---- per-cell weight blocks ----
    # pair p = 2 * t(row) + t(col); window type: 0 for w<NWT else 1
    wblk = np.zeros((C, NCELL, 128, WCOLS), BF16)
    w1c_b = np.concatenate([ew1[:, 2 * F:], eb1[:, None, :]], axis=1)  # [P,17,H]
    for c in range(C):
        for k in range(NCELL):
            tw = 0 if (k // 2) < NWT else 1
            p = 2 * tw + (k % 2)
            wblk[c, k, :, 0:H] = ew1[p, :F].astype(BF16)
            wblk[c, k, :, H:2 * H] = ew1[p, F:2 * F].astype(BF16)
            wblk[c, k, :, 2 * H:3 * H] = ew2[p].astype(BF16)
            wblk[c, k, :KEA, 3 * H:4 * H] = w1c_b[p].astype(BF16)

    # ---- per-core node tensors ----
    x0T = np.zeros((C, 128, LCAP), np.float32)
    x0Tbf = np.zeros((C, 128, LCAP), BF16)
    maskbf = np.zeros((C, 128, LCAP), np.uint8)
    for c in range(C):
        valid = node_at[c] >= 0
        xs = np.zeros((LCAP, F), np.float32)
        xs[valid] = x0[node_at[c][valid]]
        x0T[c] = xs.T
        x0Tbf[c] = xs.T.astype(BF16)
        m = np.zeros(LCAP, np.uint8)
        m[valid] = nt[node_at[c][valid]]
        maskbf[c] = np.broadcast_to(m, (128, LCAP))

    iota = np.broadcast_to(np.arange(WIN, dtype=np.float32), (128, WIN)).astype(BF16)
    rmbT = np.ascontiguousarray(
        rmb.reshape(C, T, 128).transpose(0, 2, 1)
    ).astype(np.float32)  # [C, 128, T]
    eaT = np.ascontiguousarray(ea_pad.transpose(0, 2, 1)).astype(BF16)  # [C,KEA,ECAP]

    per_core = []
    for c in range(C):
        per_core.append({
            "xtab": x0.astype(BF16),
            "idxr": _wrap_idx(row_idx[c]),
            "idxc": _wrap_idx(col_idx[c]),
            "eaT": eaT[c],
            "rmbT": rmbT[c],
            "wstream": wblk[c],
            "iota": np.ascontiguousarray(iota),
            "x0T": x0T[c],
            "x0bf": x0Tbf[c],
            "maskbf": maskbf[c],
            "nw1x": np.ascontiguousarray(nw1[:, :F].transpose(1, 0, 2)).astype(BF16),
            "nw1a": np.ascontiguousarray(nw1[:, F:].transpose(1, 0, 2)).astype(BF16),
            "nw2": np.ascontiguousarray(nw2.transpose(1, 0, 2)).astype(BF16),
            "nb1f": np.ascontiguousarray(nb1.T).astype(np.float32),
            "nb2f": np.ascontiguousarray(nb2.T).astype(np.float32),
        })
    return per_core, node_at


def _build(reps: int = 1):
    """Construct + trace the Bass module (cached)."""
    key = ("nc", reps)
    if key in _STATE:
        return _STATE[key]
    import concourse.bacc as bacc
    import concourse.mybir as mybir
    import concourse.tile as tile
    from concourse import library_config

    fp32, bf16, i16 = mybir.dt.float32, mybir.dt.bfloat16, mybir.dt.int16
    nc = bacc.Bacc("TRN2", target_bir_lowering=False)

    d_xtab = nc.dram_tensor("xtab", [N, F], bf16, kind="ExternalInput")
    d_idxr = nc.dram_tensor("idxr", [128, SIDX], i16, kind="ExternalInput")
    d_idxc = nc.dram_tensor("idxc", [128, SIDX], i16, kind="ExternalInput")
    d_eaT = nc.dram_tensor("eaT", [KEA, ECAP], bf16, kind="ExternalInput")
    d_rmbT = nc.dram_tensor("rmbT", [128, T], fp32, kind="ExternalInput")
    d_wstr = nc.dram_tensor("wstream", [NCELL, 128, WCOLS], bf16, kind="ExternalInput")
    d_iota = nc.dram_tensor("iota", [128, WIN], bf16, kind="ExternalInput")
    d_x0T = nc.dram_tensor("x0T", [128, LCAP], fp32, kind="ExternalInput")
    d_x0bf = nc.dram_tensor("x0bf", [128, LCAP], bf16, kind="ExternalInput")
    d_mask = nc.dram_tensor("maskbf", [128, LCAP], mybir.dt.uint8, kind="ExternalInput")
    d_nw1x = nc.dram_tensor("nw1x", [F, 2, H], bf16, kind="ExternalInput")
    d_nw1a = nc.dram_tensor("nw1a", [H, 2, H], bf16, kind="ExternalInput")
    d_nw2 = nc.dram_tensor("nw2", [H, 2, OUT], bf16, kind="ExternalInput")
    d_nb1 = nc.dram_tensor("nb1f", [H, 2], fp32, kind="ExternalInput")
    d_nb2 = nc.dram_tensor("nb2f", [OUT, 2], fp32, kind="ExternalInput")
    d_out = nc.dram_tensor("outT", [128, LCAP], fp32, kind="ExternalOutput")

    RELU = mybir.ActivationFunctionType.Relu
    EQ = mybir.AluOpType.is_equal

    with tile.TileContext(nc) as tc:
        with (
            tc.tile_pool(name="const", bufs=1) as cst,
            tc.tile_pool(name="wblk", bufs=3) as wpool,
            tc.tile_pool(name="eap", bufs=4) as eap,
            tc.tile_pool(name="gat", bufs=3) as gat,
            tc.tile_pool(name="h1p", bufs=3) as h1p,
            tc.tile_pool(name="efp", bufs=16) as efp,
            tc.tile_pool(name="sp", bufs=6) as sp,
            tc.tile_pool(name="nodep", bufs=1) as nodep,
            tc.tile_pool(name="pA", bufs=3, space="PSUM") as pA,
            tc.tile_pool(name="pB", bufs=3, space="PSUM") as pB,
            tc.tile_pool(name="pG", bufs=2, space="PSUM") as pG,
        ):
            idxr = cst.tile([128, SIDX], i16, tag="idxr")
            idxc = cst.tile([128, SIDX], i16, tag="idxc")
            rmbT = cst.tile([128, T], fp32, tag="rmbT")
            iota = cst.tile([128, WIN], bf16, tag="iota")
            x0T = cst.tile([128, LCAP], fp32, tag="x0T")
            x0bf = cst.tile([128, LCAP], bf16, tag="x0bf")
            mask = cst.tile([128, LCAP], mybir.dt.uint8, tag="mask")
            nw1x = cst.tile([F, 2, H], bf16, tag="nw1x")
            nw1a = cst.tile([H, 2, H], bf16, tag="nw1a")
            nw2 = cst.tile([H, 2, OUT], bf16, tag="nw2")
            nb1 = cst.tile([H, 2], fp32, tag="nb1")
            nb2 = cst.tile([OUT, 2], fp32, tag="nb2")
            aggbf = cst.tile([128, LCAP], bf16, tag="aggbf")
            outT = cst.tile([128, LCAP], fp32, tag="outT")

            for t_sb, t_dr in (
                (idxr, d_idxr), (idxc, d_idxc), (rmbT, d_rmbT),
                (iota, d_iota), (x0T, d_x0T), (x0bf, d_x0bf), (mask, d_mask),
                (nw1x, d_nw1x), (nw1a, d_nw1a), (nw2, d_nw2),
                (nb1, d_nb1), (nb2, d_nb2),
            ):
                nc.sync.dma_start(t_sb[:], t_dr[:])

            nc.gpsimd.load_library(library_config.mlp)

            import contextlib
            loop_ctx = tc.For_i(0, reps, 1) if reps > 1 else contextlib.nullcontext()
            WG = 2 * CELL  # edge slots per window (4096)
            with loop_ctx:
              for w in range(NWIN):
                xr = gat.tile([128, 1, WG], bf16, tag="xr")
                xc = gat.tile([128, 1, WG], bf16, tag="xc")
                # SWDGE descriptor rings hold 64 descs/engine; one transpose
                # gather needs num_idxs/16 + 2, so chunk calls at <= 896 idxs.
                for off in range(0, WG, 896):
                    n = min(896, WG - off)
                    isl = slice((w * WG + off) // 16, (w * WG + off + n) // 16)
                    nc.gpsimd.dma_gather(
                        xr[:, :, off:off + n], d_xtab[:], idxr[:, isl],
                        n, n, F, transpose=True)
                    nc.gpsimd.dma_gather(
                        xc[:, :, off:off + n], d_xtab[:], idxc[:, isl],
                        n, n, F, transpose=True)
                pagg = pG.tile([128, WIN], fp32, tag="pagg")
                for cc in range(2):
                    k = w * 2 + cc
                    wb = wpool.tile([128, WCOLS], bf16, tag="wb")
                    nc.sync.dma_start(wb[:], d_wstr[k])
                    eab = eap.tile([KEA, CELL], bf16, tag="eab")
                    nc.sync.dma_start(eab[:], d_eaT[:, k * CELL:(k + 1) * CELL])
                    W1a, W1b = wb[:, 0:H], wb[:, H:2 * H]
                    W2 = wb[:, 2 * H:3 * H]
                    W1c = wb[0:KEA, 3 * H:4 * H]
                    h1 = h1p.tile([128, CELL], bf16, tag="h1")
                    for g in range(CELL // 512):
                        lsl = slice(cc * CELL + g * 512, cc * CELL + (g + 1) * 512)
                        p1 = pA.tile([128, 512], fp32, tag="p1")
                        nc.tensor.matmul(p1[:], W1a, xr[:, 0, lsl],
                                         start=True, stop=False)
                        nc.tensor.matmul(p1[:], W1b, xc[:, 0, lsl],
                                         start=False, stop=False)
                        nc.tensor.matmul(
                            p1[:], W1c, eab[:, g * 512:(g + 1) * 512],
                            start=False, stop=True)
                        nc.scalar.activation(
                            h1[:, g * 512:(g + 1) * 512], p1[:], RELU)
                    for q4 in range(Q // 4):
                        # L2 for 4 tiles into one PSUM bank, one batched relu
                        p2 = pB.tile([128, 512], fp32, tag="p2")
                        for j in range(4):
                            q = q4 * 4 + j
                            nc.tensor.matmul(
                                p2[:, j * 128:(j + 1) * 128],
                                h1[:, q * 128:(q + 1) * 128],
                                W2, start=True, stop=True)
                        ef = efp.tile([128, 512], bf16, tag="ef")
                        if q4 % 2 == 0:
                            nc.scalar.activation(ef[:], p2[:], RELU)
                        else:
                            nc.vector.tensor_scalar_max(ef[:], p2[:], 0.0)
                        for j in range(4):
                            q = q4 * 4 + j
                            tglob = k * Q + q
                            S = sp.tile([128, WIN], bf16, tag="S")
                            nc.vector.tensor_scalar(
                                S[:], iota[:], rmbT[:, tglob:tglob + 1], None, EQ)
                            nc.tensor.matmul(
                                pagg[:], ef[:, j * 128:(j + 1) * 128], S[:],
                                start=(cc == 0 and q == 0),
                                stop=(cc == 1 and q == Q - 1))
                nc.vector.tensor_copy(
                    aggbf[:, w * WIN:(w + 1) * WIN], pagg[:])

              n2t = []
              for t in range(2):
                n1 = nodep.tile([128, LCAP], bf16, tag=f"n1_{t}")
                n2 = nodep.tile([128, LCAP], fp32, tag=f"n2_{t}")
                for g in range(LCAP // 512):
                    sl = slice(g * 512, (g + 1) * 512)
                    p1 = pA.tile([128, 512], fp32, tag="p1")
                    nc.tensor.matmul(p1[:], nw1x[:, t, :], x0bf[:, sl],
                                     start=True, stop=False)
                    nc.tensor.matmul(p1[:], nw1a[:, t, :], aggbf[:, sl],
                                     start=False, stop=True)
                    nc.scalar.activation(n1[:, sl], p1[:], RELU, bias=nb1[:, t:t + 1])
                for g in range(LCAP // 512):
                    sl = slice(g * 512, (g + 1) * 512)
                    p2 = pA.tile([128, 512], fp32, tag="p1")
                    nc.tensor.matmul(p2[:], nw2[:, t, :], n1[:, sl],
                                     start=True, stop=True)
                    nc.vector.tensor_scalar_add(n2[:, sl], p2[:], nb2[:, t:t + 1])
                n2t.append(n2)

              nc.vector.tensor_copy(outT[:], n2t[0][:])
              nc.vector.copy_predicated(outT[:], mask[:], n2t[1][:])
              nc.vector.tensor_tensor(
                  outT[:], outT[:], x0T[:], mybir.AluOpType.add)
              nc.sync.dma_start(d_out[:], outT[:])

    nc.compile()
    _STATE[key] = nc
    return nc


def kernel(**inputs) -> np.ndarray:
    from concourse.bass_utils import run_bass_kernel_spmd

    nc = _build()
    per_core, node_at = _prep(inputs)
    res = run_bass_kernel_spmd(nc, per_core, core_ids=list(range(C)))
    out = np.zeros((N, 1, OUT), np.float32)
    for c in range(C):
        outT = res.results[c]["outT"]  # [128, LCAP]
        valid = node_at[c] >= 0
        out[node_at[c][valid], 0, :] = outT[:, valid].T
    return out
